# revision 30
# baseline (speedup 1.0000x reference)
"""Trainium2 Bass kernel for nn_MultiHeadDiffAttention (B=2,T=2048,C=1024,H=16).

Sharding: 8 cores = data-parallel over B(2) x tensor-parallel over 4 head-groups
(4 heads each). Each core computes q/k/v projections for its heads, causal
differential attention, per-head GroupNorm, and a partial output projection
(its 512 columns of y2 against Wc). Host sums the 4 partials per batch.

Layout strategy per core:
  - x is passed transposed+bf16 from host: xT [C=1024, T=2048].
  - qT/kT per head [64, T] via matmul(lhsT=W^T chunk, rhs=xT chunk).
  - v in [t, d] layout via matmul(lhsT=xT chunk, rhs=WvT).
  - S tiles [q=128, k<=512] on PE (bf16), causal-trimmed; diagonal 128-blocks
    masked via an additive -30000 mask before exp.
  - ACT exp with accum_out gives the softmax denominators for free; the
    normalization (1/D1, -lam/D2) is folded into the attention weights with
    per-partition scalars on DVE (2 passes), producing combined
    att = att1 - lam*att2 directly.
  - att tiles are PE-transposed to [k, q] (bf16), then z = att @ v accumulates
    on PE into [q, d] per q-tile.
  - z transposed to yT [d, t]; GroupNorm stats via free-dim reduces + a tiny
    fp32 matmul against a group-indicator matrix; affine applied in one ACT
    pass (scale/bias per partition) -> yT bf16.
  - partial out^T [o=1024, t] = Wc_slice^T.T @ yT accumulated over the 4 heads.
Host gathers: out[b] = sum over head-group cores of outT.T.
"""

import sys

for _p in ("/opt/trn_rl_repo", "/root/.axon_site/_ro/trn_rl_repo"):
    if _p not in sys.path:
        sys.path.insert(0, _p)

import math
import numpy as np
import ml_dtypes

import concourse.bass as bass
import concourse.bacc as bacc
import concourse.tile as tile
import concourse.mybir as mybir
from concourse import bass_utils

F32 = mybir.dt.float32
BF16 = mybir.dt.bfloat16
AF = mybir.ActivationFunctionType
ALU = mybir.AluOpType

B, T, C = 2, 2048, 1024
H = 16
HS = C // H           # 64
D = 2 * HS            # 128 v-channels per head
NH = 4                # heads per core
N_CORES = 8
NT = T // 128         # 16 q-tiles
LAMBDA_INIT = 0.8 - 0.6 * math.exp(-0.3 * (12 - 1))
EPS = 1e-5
SCALE = 1.0 / math.sqrt(HS)
NEG = -30000.0

_cache = {}


def _build(T=T, trace_sim=False, stage=5, nh=NH):
    # stage: 1=proj only, 2=+S/exp/combine, 3=+transpose+z, 4=+groupnorm,
    #        5=full (out-proj). nh: number of heads to process (debug).
    NT = T // 128
    nc = bacc.Bacc("TRN2", target_bir_lowering=False, debug=False,
                   num_devices=N_CORES)

    def din(name, shape, dt=BF16):
        return nc.dram_tensor(name, shape, dt, kind="ExternalInput").ap()

    xT_d = din("xT", [C, T])
    wq1_d = din("wq1T", [C, NH * HS])
    wq2_d = din("wq2T", [C, NH * HS])
    wk1_d = din("wk1T", [C, NH * HS])
    wk2_d = din("wk2T", [C, NH * HS])
    wv_d = din("wvT", [C, NH * D])
    wc_d = din("wcT", [NH * D, C])
    mask_d = din("mask128", [128, 128], F32)
    ident_d = din("ident", [128, 128])
    gg_d = din("gg", [128, 128], F32)
    gw2_d = din("gw2", [128, 1], F32)
    gb2_d = din("gb2", [128, 1], F32)
    lamn_d = din("lamn", [128, NH], F32)
    outT_d = nc.dram_tensor("outT", [C, T], F32, kind="ExternalOutput").ap()
    dbg_d = (nc.dram_tensor("dbg", [128, T], F32, kind="ExternalOutput").ap()
             if stage != 5 else None)

    with tile.TileContext(nc, trace_sim=trace_sim) as tc:
        with tc.tile_pool(name="persist", bufs=1) as pp, \
             tc.tile_pool(name="ps_s", bufs=2, space="PSUM") as ps_s, \
             tc.tile_pool(name="ps_t", bufs=2, space="PSUM") as ps_t, \
             tc.tile_pool(name="ps_z", bufs=2, space="PSUM") as ps_z:

            # ---- persistent small tiles ----
            mask_t = pp.tile([128, 128], F32, tag="mask")
            nc.sync.dma_start(mask_t[:], mask_d)
            ident_t = pp.tile([128, 128], BF16, tag="ident")
            nc.sync.dma_start(ident_t[:], ident_d)
            gg_t = pp.tile([128, 128], F32, tag="gg")
            nc.sync.dma_start(gg_t[:], gg_d)
            gw2_t = pp.tile([128, 1], F32, tag="gw2")
            nc.sync.dma_start(gw2_t[:], gw2_d)
            gb2_t = pp.tile([128, 1], F32, tag="gb2")
            nc.sync.dma_start(gb2_t[:], gb2_d)
            lamn_t = pp.tile([128, NH], F32, tag="lamn")
            nc.sync.dma_start(lamn_t[:], lamn_d)

            # ---- persistent activation tensors ----
            # qT/kT: [NH*HS=256, T] as 2 partition-tiles of 128
            q1t = [pp.tile([128, T], BF16, tag=f"q1t{i}", name=f"q1t{i}") for i in range(2)]
            q2t = [pp.tile([128, T], BF16, tag=f"q2t{i}", name=f"q2t{i}") for i in range(2)]
            k1t = [pp.tile([128, T], BF16, tag=f"k1t{i}", name=f"k1t{i}") for i in range(2)]
            k2t = [pp.tile([128, T], BF16, tag=f"k2t{i}", name=f"k2t{i}") for i in range(2)]
            # v: [T, NH*D=512] as 16 t-chunk tiles
            vt = [pp.tile([128, NH * D], BF16, tag=f"vt{i}", name=f"vt{i}") for i in range(NT)]
            # yT per head [D=128, T] bf16 (post-groupnorm)
            yt = [pp.tile([128, T], BF16, tag=f"yt{j}", name=f"yt{j}") for j in range(NH)]
            # wcT: [512, C] as 4 f-chunk tiles (one per head)
            wct = [pp.tile([128, C], BF16, tag=f"wct{j}", name=f"wct{j}") for j in range(NH)]
            for j in range(NH):
                nc.sync.dma_start(wct[j][:], wc_d[j * 128:(j + 1) * 128, :])

            # ================= projections =================
            with tc.tile_pool(name="loads", bufs=1) as lp:
                xt = [lp.tile([128, T], BF16, tag=f"xt{i}", name=f"xt{i}") for i in range(8)]
                for i in range(8):
                    nc.sync.dma_start(xt[i][:], xT_d[i * 128:(i + 1) * 128, :])
                wq = {}
                for nm, d_ap in (("q1", wq1_d), ("q2", wq2_d),
                                 ("k1", wk1_d), ("k2", wk2_d)):
                    wq[nm] = [lp.tile([128, NH * HS], BF16, tag=f"w{nm}{i}", name=f"w{nm}{i}")
                              for i in range(8)]
                    for i in range(8):
                        nc.sync.dma_start(wq[nm][i][:],
                                          d_ap[i * 128:(i + 1) * 128, :])
                wvt = [lp.tile([128, NH * D], BF16, tag=f"wvt{i}", name=f"wvt{i}")
                       for i in range(8)]
                for i in range(8):
                    nc.sync.dma_start(wvt[i][:], wv_d[i * 128:(i + 1) * 128, :])

                # qT/kT projections: out [o=128, t=512] = W^T_chunk.T @ xT
                for nm, dst in (("q1", q1t), ("q2", q2t),
                                ("k1", k1t), ("k2", k2t)):
                    for oc in range(2):
                        for tb in range(T // 512):
                            ps = ps_s.tile([128, 512], F32, tag="s")
                            for cc in range(8):
                                nc.tensor.matmul(
                                    ps[:],
                                    wq[nm][cc][:, oc * 128:(oc + 1) * 128],
                                    xt[cc][:, tb * 512:(tb + 1) * 512],
                                    start=(cc == 0), stop=(cc == 7))
                            eng = nc.scalar if nm in ("q1", "k1") else nc.vector
                            if eng is nc.scalar:
                                nc.scalar.copy(
                                    dst[oc][:, tb * 512:(tb + 1) * 512], ps[:])
                            else:
                                nc.vector.tensor_copy(
                                    dst[oc][:, tb * 512:(tb + 1) * 512], ps[:])

                # v projection: out [t=128, d=512] = xT_chunk.T @ WvT
                for tch in range(NT):
                    ps = ps_s.tile([128, 512], F32, tag="s")
                    for cc in range(8):
                        nc.tensor.matmul(
                            ps[:],
                            xt[cc][:, tch * 128:(tch + 1) * 128],
                            wvt[cc][:],
                            start=(cc == 0), stop=(cc == 7))
                    nc.scalar.copy(vt[tch][:], ps[:])

            # ================= attention per head =================
            wp_cm = tc.tile_pool(name="aw", bufs=2)
            wp = wp_cm.__enter__()
            if stage == 1:
                dbg_t = wp.tile([128, T], F32, tag="dbg_t")
                nc.vector.tensor_copy(dbg_t[:], q1t[0][:])
                nc.sync.dma_start(dbg_d, dbg_t[:])
            for j in range(nh if stage >= 2 else 0):
                oc, po = divmod(j * HS, 128)   # which qT/kT tile + part offset
                ytr = wp.tile([128, T], BF16, tag="ytr")  # yT raw [d, t]
                s1p = wp.tile([128, 4], F32, tag="s1p")
                s2p = wp.tile([128, 4], F32, tag="s2p")
                for qb in range(NT // 4):
                    att_rows = {}
                    for qq in range(4):
                        qt = qb * 4 + qq
                        nk = qt + 1
                        nkb2 = (nk + 7) // 8   # 1024-wide S psum tiles
                        e1 = wp.tile([128, T], BF16, tag="e1", bufs=3)
                        e2 = wp.tile([128, T], BF16, tag="e2", bufs=3)
                        d1c = wp.tile([128, 2], F32, tag="d1c")
                        d2c = wp.tile([128, 2], F32, tag="d2c")
                        for mi, (qsrc, ksrc, erow, dcol) in enumerate(
                                ((q1t, k1t, e1, d1c), (q2t, k2t, e2, d2c))):
                            for kb in range(nkb2):
                                w = min(1024, nk * 128 - kb * 1024)
                                ps = ps_s.tile([128, 1024], F32, tag="s")
                                for hf in range(2):
                                    wh = min(512, w - hf * 512)
                                    if wh <= 0:
                                        break
                                    nc.tensor.matmul(
                                        ps[:, hf * 512:hf * 512 + wh],
                                        qsrc[oc][po:po + HS,
                                                 qt * 128:(qt + 1) * 128],
                                        ksrc[oc][po:po + HS,
                                                 kb * 1024 + hf * 512:
                                                 kb * 1024 + hf * 512 + wh],
                                        start=True, stop=True)
                                if kb == nkb2 - 1:
                                    # mask diagonal 128-block (k-chunk qt)
                                    off = qt * 128 - kb * 1024
                                    nc.vector.tensor_tensor(
                                        ps[:, off:off + 128],
                                        ps[:, off:off + 128],
                                        mask_t[:], ALU.add)
                                nc.scalar.activation(
                                    erow[:, kb * 1024:kb * 1024 + w],
                                    ps[:, :w], AF.Exp, scale=SCALE,
                                    accum_out=dcol[:, kb:kb + 1])
                        # denominators -> r1, r2n = -lam/D2
                        dd = wp.tile([128, 2], F32, tag="dd")
                        nc.vector.tensor_reduce(dd[:, 0:1], d1c[:, 0:nkb2],
                                                axis=mybir.AxisListType.X,
                                                op=ALU.add)
                        nc.vector.tensor_reduce(dd[:, 1:2], d2c[:, 0:nkb2],
                                                axis=mybir.AxisListType.X,
                                                op=ALU.add)
                        rr = wp.tile([128, 2], F32, tag="rr")
                        nc.vector.reciprocal(rr[:], dd[:, 0:2])
                        r2n = wp.tile([128, 1], F32, tag="r2n")
                        nc.vector.tensor_tensor(r2n[:], rr[:, 1:2],
                                                lamn_t[:, j:j + 1], ALU.mult)
                        # combined normalized att = e1*r1 - lam*r2*e2 (bf16)
                        e2s = wp.tile([128, T], BF16, tag="e2s")
                        nc.vector.tensor_scalar_mul(e2s[:, :nk * 128],
                                                    e2[:, :nk * 128], r2n[:])
                        att = wp.tile([128, T], BF16, tag=f"att{qq}",
                                      name=f"att{qq}")
                        nc.vector.scalar_tensor_tensor(
                            att[:, :nk * 128], e1[:, :nk * 128], rr[:, 0:1],
                            e2s[:, :nk * 128], op0=ALU.mult, op1=ALU.add)
                        att_rows[qt] = att
                    if stage == 2:
                        if j == 0 and qb == NT // 4 - 1:
                            dbg_t = wp.tile([128, T], F32, tag="dbg_t")
                            nc.vector.tensor_copy(dbg_t[:],
                                                  att_rows[NT - 1][:])
                            nc.sync.dma_start(dbg_d, dbg_t[:])
                        continue
                    # transposes grouped by k-chunk -> attT blocks [k, qblk]
                    nkc = qb * 4 + 4
                    ablk = []
                    for kc in range(nkc):
                        pt = ps_t.tile([128, 512], BF16, tag="t")
                        for qq in range(4):
                            qt = qb * 4 + qq
                            if qt >= kc:
                                nc.tensor.transpose(
                                    pt[:, qq * 128:qq * 128 + 128],
                                    att_rows[qt][:, kc * 128:kc * 128 + 128],
                                    ident_t[:])
                        ab = wp.tile([128, 512], BF16, tag=f"atb{kc}",
                                     name=f"atb{kc}")
                        zw = max(0, (kc - qb * 4) * 128)
                        nc.vector.tensor_copy(ab[:, zw:], pt[:, zw:])
                        ablk.append((ab, zw))
                    # yT[d, qblk] = sum_kc v_kc.T @ attT_kc   (N=512)
                    py = ps_z.tile([128, 512], F32, tag="z")
                    for kc in range(nkc):
                        ab, zw = ablk[kc]
                        nc.tensor.matmul(
                            py[:, zw:],
                            vt[kc][:, j * 128:(j + 1) * 128],
                            ab[:, zw:],
                            start=(kc == 0), stop=(kc == nkc - 1),
                            skip_group_check=True)
                    # copy to ytr with fused stats accumulation (ACT)
                    nc.scalar.activation(
                        ytr[:, qb * 512:(qb + 1) * 512], py[:], AF.Copy,
                        accum_out=s1p[:, qb:qb + 1])
                    ysq = wp.tile([128, 512], BF16, tag="ysq", bufs=1)
                    nc.scalar.activation(
                        ysq[:], py[:], AF.Square,
                        accum_out=s2p[:, qb:qb + 1])

                if stage == 2:
                    continue
                if stage == 3:
                    if j == 0:
                        dbg_t = wp.tile([128, T], F32, tag="dbg_t")
                        nc.vector.tensor_copy(dbg_t[:], ytr[:])
                        nc.sync.dma_start(dbg_d, dbg_t[:])
                    continue
                # ---- GroupNorm stats ----
                if stage == 41:
                    if j == 0:
                        dbg_t = wp.tile([128, T], F32, tag="dbg_t")
                        nc.vector.tensor_copy(dbg_t[:], ytr[:])
                        nc.sync.dma_start(dbg_d, dbg_t[:])
                    continue
                s12 = wp.tile([128, 2], F32, tag="s12")
                nc.vector.tensor_reduce(s12[:, 0:1], s1p[:, 0:NT // 4],
                                        axis=mybir.AxisListType.X, op=ALU.add)
                nc.vector.tensor_reduce(s12[:, 1:2], s2p[:, 0:NT // 4],
                                        axis=mybir.AxisListType.X, op=ALU.add)
                if stage == 42:
                    if j == 0:
                        dbg_t = wp.tile([128, T], F32, tag="dbg_t")
                        nc.vector.tensor_copy(dbg_t[:, 0:2], s12[:])
                        nc.sync.dma_start(dbg_d[:, 0:2], dbg_t[:, 0:2])
                    continue
                pg = ps_z.tile([128, 2], F32, tag="z")
                nc.tensor.matmul(pg[:], gg_t[:], s12[:], start=True, stop=True)
                if stage == 43:
                    if j == 0:
                        dbg_t = wp.tile([128, T], F32, tag="dbg_t")
                        nc.vector.tensor_copy(dbg_t[:, 0:2], pg[:])
                        nc.sync.dma_start(dbg_d[:, 0:2], dbg_t[:, 0:2])
                    continue
                # mneg = -mean; nvar = mean^2 - E[y^2] = -var
                mneg = wp.tile([128, 1], F32, tag="mneg")
                nc.scalar.mul(mneg[:], pg[:, 0:1], -1.0 / (T * 4))
                msq = wp.tile([128, 1], F32, tag="msq")
                nc.scalar.mul(msq[:], pg[:, 1:2], 1.0 / (T * 4))
                nvar = wp.tile([128, 1], F32, tag="nvar")
                nc.vector.scalar_tensor_tensor(
                    nvar[:], mneg[:], mneg[:, 0:1], msq[:],
                    op0=ALU.mult, op1=ALU.subtract)
                vpe = wp.tile([128, 1], F32, tag="vpe")
                nc.vector.tensor_scalar(vpe[:], nvar[:], -1.0, EPS,
                                        op0=ALU.mult, op1=ALU.add)  # var+eps
                std = wp.tile([128, 1], F32, tag="std")
                nc.scalar.activation(std[:], vpe[:], AF.Sqrt)
                rstd = wp.tile([128, 1], F32, tag="rstd")
                nc.vector.reciprocal(rstd[:], std[:])
                aff_a = wp.tile([128, 1], F32, tag="aff_a")
                nc.vector.tensor_tensor(aff_a[:], rstd[:], gw2_t[:], ALU.mult)
                aff_b = wp.tile([128, 1], F32, tag="aff_b")
                nc.vector.scalar_tensor_tensor(
                    aff_b[:], mneg[:], aff_a[:, 0:1], gb2_t[:],
                    op0=ALU.mult, op1=ALU.add)  # gb2 - mean*aff_a
                nc.scalar.activation(yt[j][:], ytr[:], AF.Identity,
                                     scale=aff_a[:], bias=aff_b[:])
                if stage == 4 and j == 0:
                    dbg_t = wp.tile([128, T], F32, tag="dbg_t")
                    nc.vector.tensor_copy(dbg_t[:], yt[0][:])
                    nc.sync.dma_start(dbg_d, dbg_t[:])

            # ================= output projection =================
            for ocb in range(8 if stage == 5 else 0):
                for tb in range(T // 512):
                    po_ = ps_z.tile([128, 512], F32, tag="z")
                    for j in range(NH):
                        nc.tensor.matmul(
                            po_[:],
                            wct[j][:, ocb * 128:(ocb + 1) * 128],
                            yt[j][:, tb * 512:(tb + 1) * 512],
                            start=(j == 0), stop=(j == NH - 1))
                    ob = wp.tile([128, 512], F32, tag="ob")
                    nc.vector.tensor_copy(ob[:], po_[:])
                    nc.sync.dma_start(
                        outT_d[ocb * 128:(ocb + 1) * 128,
                               tb * 512:(tb + 1) * 512], ob[:])
            wp_cm.__exit__(None, None, None)

    nc.compile()
    return nc


def _prep_inputs(inputs):
    bf = ml_dtypes.bfloat16
    x = np.asarray(inputs["x"], np.float32)
    Wq1 = np.asarray(inputs["Wq1"], np.float32)
    Wq2 = np.asarray(inputs["Wq2"], np.float32)
    Wk1 = np.asarray(inputs["Wk1"], np.float32)
    Wk2 = np.asarray(inputs["Wk2"], np.float32)
    Wv = np.asarray(inputs["Wv"], np.float32)
    Wc = np.asarray(inputs["Wc"], np.float32)
    gn_w = np.asarray(inputs["gn_w"], np.float32)
    gn_b = np.asarray(inputs["gn_b"], np.float32)
    gamma = np.asarray(inputs["gamma"], np.float32)

    def sig(v):
        return 1.0 / (1.0 + np.exp(-v))

    lam = (sig(np.asarray(inputs["lq1"], np.float32).reshape(H)
               * np.asarray(inputs["lk1"], np.float32).reshape(H))
           - sig(np.asarray(inputs["lq2"], np.float32).reshape(H)
                 * np.asarray(inputs["lk2"], np.float32).reshape(H))
           + LAMBDA_INIT)

    mask = np.where(np.arange(128)[None, :] <= np.arange(128)[:, None],
                    0.0, NEG).astype(np.float32)
    ident = np.eye(128, dtype=np.float32).astype(bf)
    gg = (np.arange(128)[:, None] // 4 == np.arange(128)[None, :] // 4
          ).astype(np.float32)
    c1 = 1.0 - LAMBDA_INIT
    gw2 = (gn_w * gamma * c1).astype(np.float32).reshape(128, 1)
    gb2 = (gn_b * gamma * c1).astype(np.float32).reshape(128, 1)

    xTb = [np.ascontiguousarray(x[b].T).astype(bf) for b in range(B)]
    in_maps = []
    for core in range(N_CORES):
        b, hg = divmod(core, N_CORES // B)
        qs = hg * NH * HS          # 256-wide q/k slice
        vs = hg * NH * D           # 512-wide v / y2 slice
        lamn = np.repeat(-lam[hg * NH:(hg + 1) * NH].reshape(1, NH),
                         128, axis=0).astype(np.float32)
        in_maps.append({
            "xT": xTb[b],
            "wq1T": np.ascontiguousarray(Wq1[qs:qs + NH * HS, :].T).astype(bf),
            "wq2T": np.ascontiguousarray(Wq2[qs:qs + NH * HS, :].T).astype(bf),
            "wk1T": np.ascontiguousarray(Wk1[qs:qs + NH * HS, :].T).astype(bf),
            "wk2T": np.ascontiguousarray(Wk2[qs:qs + NH * HS, :].T).astype(bf),
            "wvT": np.ascontiguousarray(Wv[vs:vs + NH * D, :].T).astype(bf),
            "wcT": np.ascontiguousarray(Wc[:, vs:vs + NH * D].T).astype(bf),
            "mask128": mask,
            "ident": ident,
            "gg": gg,
            "gw2": gw2,
            "gb2": gb2,
            "lamn": lamn,
        })
    return in_maps


def kernel(**inputs):
    if "nc" not in _cache:
        _cache["nc"] = _build()
    nc = _cache["nc"]
    in_maps = _prep_inputs(inputs)
    res = bass_utils.run_bass_kernel_spmd(
        nc, in_maps, core_ids=list(range(N_CORES)),
        **_cache.get("run_kwargs", {}))
    _cache["last_result"] = res
    out = np.zeros((B, T, C), np.float32)
    for core in range(N_CORES):
        b = core // (N_CORES // B)
        out[b] += res.results[core]["outT"].T
    return out


# revision 31
# speedup vs baseline: 1.0211x; 1.0211x over previous
"""Trainium2 Bass kernel for nn_MultiHeadDiffAttention (B=2,T=2048,C=1024,H=16).

Sharding: 8 cores = data-parallel over B(2) x tensor-parallel over 4 head-groups
(4 heads each). Each core computes q/k/v projections for its heads, causal
differential attention, per-head GroupNorm, and a partial output projection
(its 512 columns of y2 against Wc). Host sums the 4 partials per batch.

Layout strategy per core:
  - x is passed transposed+bf16 from host: xT [C=1024, T=2048].
  - qT/kT per head [64, T] via matmul(lhsT=W^T chunk, rhs=xT chunk).
  - v in [t, d] layout via matmul(lhsT=xT chunk, rhs=WvT).
  - S tiles [q=128, k<=512] on PE (bf16), causal-trimmed; diagonal 128-blocks
    masked via an additive -30000 mask before exp.
  - ACT exp with accum_out gives the softmax denominators for free; the
    normalization (1/D1, -lam/D2) is folded into the attention weights with
    per-partition scalars on DVE (2 passes), producing combined
    att = att1 - lam*att2 directly.
  - att tiles are PE-transposed to [k, q] (bf16), then z = att @ v accumulates
    on PE into [q, d] per q-tile.
  - z transposed to yT [d, t]; GroupNorm stats via free-dim reduces + a tiny
    fp32 matmul against a group-indicator matrix; affine applied in one ACT
    pass (scale/bias per partition) -> yT bf16.
  - partial out^T [o=1024, t] = Wc_slice^T.T @ yT accumulated over the 4 heads.
Host gathers: out[b] = sum over head-group cores of outT.T.
"""

import sys

for _p in ("/opt/trn_rl_repo", "/root/.axon_site/_ro/trn_rl_repo"):
    if _p not in sys.path:
        sys.path.insert(0, _p)

import math
import numpy as np
import ml_dtypes

import concourse.bass as bass
import concourse.bacc as bacc
import concourse.tile as tile
import concourse.mybir as mybir
from concourse import bass_utils

F32 = mybir.dt.float32
BF16 = mybir.dt.bfloat16
AF = mybir.ActivationFunctionType
ALU = mybir.AluOpType

B, T, C = 2, 2048, 1024
H = 16
HS = C // H           # 64
D = 2 * HS            # 128 v-channels per head
NH = 4                # heads per core
N_CORES = 8
NT = T // 128         # 16 q-tiles
LAMBDA_INIT = 0.8 - 0.6 * math.exp(-0.3 * (12 - 1))
EPS = 1e-5
SCALE = 1.0 / math.sqrt(HS)
NEG = -30000.0

_cache = {}


def _build(T=T, trace_sim=False, stage=5, nh=NH):
    # stage: 1=proj only, 2=+S/exp/combine, 3=+transpose+z, 4=+groupnorm,
    #        5=full (out-proj). nh: number of heads to process (debug).
    NT = T // 128
    nc = bacc.Bacc("TRN2", target_bir_lowering=False, debug=False,
                   num_devices=N_CORES)

    def din(name, shape, dt=BF16):
        return nc.dram_tensor(name, shape, dt, kind="ExternalInput").ap()

    xT_d = din("xT", [C, T])
    wq1_d = din("wq1T", [C, NH * HS])
    wq2_d = din("wq2T", [C, NH * HS])
    wk1_d = din("wk1T", [C, NH * HS])
    wk2_d = din("wk2T", [C, NH * HS])
    wv_d = din("wvT", [C, NH * D])
    wc_d = din("wcT", [NH * D, C])
    mask_d = din("mask128", [128, 128], F32)
    ident_d = din("ident", [128, 128])
    gg_d = din("gg", [128, 128], F32)
    gw2_d = din("gw2", [128, 1], F32)
    gb2_d = din("gb2", [128, 1], F32)
    lamn_d = din("lamn", [128, NH], F32)
    outT_d = nc.dram_tensor("outT", [C, T], F32, kind="ExternalOutput").ap()
    dbg_d = (nc.dram_tensor("dbg", [128, T], F32, kind="ExternalOutput").ap()
             if stage != 5 else None)

    with tile.TileContext(nc, trace_sim=trace_sim) as tc:
        with tc.tile_pool(name="persist", bufs=1) as pp, \
             tc.tile_pool(name="ps_s", bufs=2, space="PSUM") as ps_s, \
             tc.tile_pool(name="ps_t", bufs=2, space="PSUM") as ps_t, \
             tc.tile_pool(name="ps_z", bufs=2, space="PSUM") as ps_z:

            # ---- persistent small tiles ----
            mask_t = pp.tile([128, 128], F32, tag="mask")
            nc.sync.dma_start(mask_t[:], mask_d)
            ident_t = pp.tile([128, 128], BF16, tag="ident")
            nc.sync.dma_start(ident_t[:], ident_d)
            gg_t = pp.tile([128, 128], F32, tag="gg")
            nc.sync.dma_start(gg_t[:], gg_d)
            gw2_t = pp.tile([128, 1], F32, tag="gw2")
            nc.sync.dma_start(gw2_t[:], gw2_d)
            gb2_t = pp.tile([128, 1], F32, tag="gb2")
            nc.sync.dma_start(gb2_t[:], gb2_d)
            lamn_t = pp.tile([128, NH], F32, tag="lamn")
            nc.sync.dma_start(lamn_t[:], lamn_d)

            # ---- persistent activation tensors ----
            # qT/kT: [NH*HS=256, T] as 2 partition-tiles of 128
            q1t = [pp.tile([128, T], BF16, tag=f"q1t{i}", name=f"q1t{i}") for i in range(2)]
            q2t = [pp.tile([128, T], BF16, tag=f"q2t{i}", name=f"q2t{i}") for i in range(2)]
            k1t = [pp.tile([128, T], BF16, tag=f"k1t{i}", name=f"k1t{i}") for i in range(2)]
            k2t = [pp.tile([128, T], BF16, tag=f"k2t{i}", name=f"k2t{i}") for i in range(2)]
            # v: [T, NH*D=512] as 16 t-chunk tiles
            vt = [pp.tile([128, NH * D], BF16, tag=f"vt{i}", name=f"vt{i}") for i in range(NT)]
            # yT per head [D=128, T] bf16 (post-groupnorm)
            yt = [pp.tile([128, T], BF16, tag=f"yt{j}", name=f"yt{j}") for j in range(NH)]
            # wcT: [512, C] as 4 f-chunk tiles (one per head)
            wct = [pp.tile([128, C], BF16, tag=f"wct{j}", name=f"wct{j}") for j in range(NH)]
            for j in range(NH):
                nc.sync.dma_start(wct[j][:], wc_d[j * 128:(j + 1) * 128, :])

            # ================= projections =================
            with tc.tile_pool(name="loads", bufs=1) as lp:
                xt = [lp.tile([128, T], BF16, tag=f"xt{i}", name=f"xt{i}") for i in range(8)]
                wq = {}
                for nm, d_ap in (("q1", wq1_d), ("q2", wq2_d),
                                 ("k1", wk1_d), ("k2", wk2_d)):
                    wq[nm] = [lp.tile([128, NH * HS], BF16, tag=f"w{nm}{i}", name=f"w{nm}{i}")
                              for i in range(8)]
                for i in range(8):
                    nc.sync.dma_start(wq["q1"][i][:],
                                      wq1_d[i * 128:(i + 1) * 128, :])
                    nc.sync.dma_start(xt[i][:], xT_d[i * 128:(i + 1) * 128, :])
                for nm, d_ap in (("k1", wk1_d), ("q2", wq2_d),
                                 ("k2", wk2_d)):
                    for i in range(8):
                        nc.sync.dma_start(wq[nm][i][:],
                                          d_ap[i * 128:(i + 1) * 128, :])
                wvt = [lp.tile([128, NH * D], BF16, tag=f"wvt{i}", name=f"wvt{i}")
                       for i in range(8)]
                for i in range(8):
                    nc.sync.dma_start(wvt[i][:], wv_d[i * 128:(i + 1) * 128, :])

                # qT/kT projections: out [o=128, t=512] = W^T_chunk.T @ xT
                for nm, dst in (("q1", q1t), ("q2", q2t),
                                ("k1", k1t), ("k2", k2t)):
                    for oc in range(2):
                        for tb in range(T // 512):
                            ps = ps_s.tile([128, 512], F32, tag="s")
                            for cc in range(8):
                                nc.tensor.matmul(
                                    ps[:],
                                    wq[nm][cc][:, oc * 128:(oc + 1) * 128],
                                    xt[cc][:, tb * 512:(tb + 1) * 512],
                                    start=(cc == 0), stop=(cc == 7))
                            eng = nc.scalar if nm in ("q1", "k1") else nc.vector
                            if eng is nc.scalar:
                                nc.scalar.copy(
                                    dst[oc][:, tb * 512:(tb + 1) * 512], ps[:])
                            else:
                                nc.vector.tensor_copy(
                                    dst[oc][:, tb * 512:(tb + 1) * 512], ps[:])

                # v projection: out [t=128, d=512] = xT_chunk.T @ WvT
                for tch in range(NT):
                    ps = ps_s.tile([128, 512], F32, tag="s")
                    for cc in range(8):
                        nc.tensor.matmul(
                            ps[:],
                            xt[cc][:, tch * 128:(tch + 1) * 128],
                            wvt[cc][:],
                            start=(cc == 0), stop=(cc == 7))
                    nc.scalar.copy(vt[tch][:], ps[:])

            # ================= attention per head =================
            wp_cm = tc.tile_pool(name="aw", bufs=2)
            wp = wp_cm.__enter__()
            if stage == 1:
                dbg_t = wp.tile([128, T], F32, tag="dbg_t")
                nc.vector.tensor_copy(dbg_t[:], q1t[0][:])
                nc.sync.dma_start(dbg_d, dbg_t[:])
            for j in range(nh if stage >= 2 else 0):
                oc, po = divmod(j * HS, 128)   # which qT/kT tile + part offset
                ytr = wp.tile([128, T], BF16, tag="ytr")  # yT raw [d, t]
                s1p = wp.tile([128, 4], F32, tag="s1p")
                s2p = wp.tile([128, 4], F32, tag="s2p")
                for qb in range(NT // 4):
                    att_rows = {}
                    for qq in range(4):
                        qt = qb * 4 + qq
                        nk = qt + 1
                        nkb2 = (nk + 7) // 8   # 1024-wide S psum tiles
                        e1 = wp.tile([128, T], BF16, tag="e1", bufs=3)
                        e2 = wp.tile([128, T], BF16, tag="e2", bufs=3)
                        d1c = wp.tile([128, 2], F32, tag="d1c")
                        d2c = wp.tile([128, 2], F32, tag="d2c")
                        for mi, (qsrc, ksrc, erow, dcol) in enumerate(
                                ((q1t, k1t, e1, d1c), (q2t, k2t, e2, d2c))):
                            for kb in range(nkb2):
                                w = min(1024, nk * 128 - kb * 1024)
                                ps = ps_s.tile([128, 1024], F32, tag="s")
                                for hf in range(2):
                                    wh = min(512, w - hf * 512)
                                    if wh <= 0:
                                        break
                                    nc.tensor.matmul(
                                        ps[:, hf * 512:hf * 512 + wh],
                                        qsrc[oc][po:po + HS,
                                                 qt * 128:(qt + 1) * 128],
                                        ksrc[oc][po:po + HS,
                                                 kb * 1024 + hf * 512:
                                                 kb * 1024 + hf * 512 + wh],
                                        start=True, stop=True)
                                if kb == nkb2 - 1:
                                    # mask diagonal 128-block (k-chunk qt)
                                    off = qt * 128 - kb * 1024
                                    nc.vector.tensor_tensor(
                                        ps[:, off:off + 128],
                                        ps[:, off:off + 128],
                                        mask_t[:], ALU.add)
                                nc.scalar.activation(
                                    erow[:, kb * 1024:kb * 1024 + w],
                                    ps[:, :w], AF.Exp, scale=SCALE,
                                    accum_out=dcol[:, kb:kb + 1])
                        # denominators -> r1, r2n = -lam/D2
                        dd = wp.tile([128, 2], F32, tag="dd")
                        nc.vector.tensor_reduce(dd[:, 0:1], d1c[:, 0:nkb2],
                                                axis=mybir.AxisListType.X,
                                                op=ALU.add)
                        nc.vector.tensor_reduce(dd[:, 1:2], d2c[:, 0:nkb2],
                                                axis=mybir.AxisListType.X,
                                                op=ALU.add)
                        rr = wp.tile([128, 2], F32, tag="rr")
                        nc.vector.reciprocal(rr[:], dd[:, 0:2])
                        r2n = wp.tile([128, 1], F32, tag="r2n")
                        nc.vector.tensor_tensor(r2n[:], rr[:, 1:2],
                                                lamn_t[:, j:j + 1], ALU.mult)
                        # combined normalized att = e1*r1 - lam*r2*e2 (bf16)
                        e2s = wp.tile([128, T], BF16, tag="e2s")
                        nc.vector.tensor_scalar_mul(e2s[:, :nk * 128],
                                                    e2[:, :nk * 128], r2n[:])
                        att = wp.tile([128, T], BF16, tag=f"att{qq}",
                                      name=f"att{qq}")
                        nc.vector.scalar_tensor_tensor(
                            att[:, :nk * 128], e1[:, :nk * 128], rr[:, 0:1],
                            e2s[:, :nk * 128], op0=ALU.mult, op1=ALU.add)
                        att_rows[qt] = att
                    if stage == 2:
                        if j == 0 and qb == NT // 4 - 1:
                            dbg_t = wp.tile([128, T], F32, tag="dbg_t")
                            nc.vector.tensor_copy(dbg_t[:],
                                                  att_rows[NT - 1][:])
                            nc.sync.dma_start(dbg_d, dbg_t[:])
                        continue
                    # transposes grouped by k-chunk -> attT blocks [k, qblk]
                    nkc = qb * 4 + 4
                    ablk = []
                    for kc2 in range((nkc + 1) // 2):
                        pt = ps_t.tile([128, 1024], BF16, tag="t")
                        for half in range(2):
                            kc = kc2 * 2 + half
                            if kc >= nkc:
                                break
                            for qq in range(4):
                                qt = qb * 4 + qq
                                if qt >= kc:
                                    nc.tensor.transpose(
                                        pt[:, half * 512 + qq * 128:
                                           half * 512 + qq * 128 + 128],
                                        att_rows[qt][:,
                                                     kc * 128:kc * 128 + 128],
                                        ident_t[:])
                            ab = wp.tile([128, 512], BF16, tag=f"atb{kc}",
                                         name=f"atb{kc}")
                            zw = max(0, (kc - qb * 4) * 128)
                            nc.vector.tensor_copy(
                                ab[:, zw:], pt[:, half * 512 + zw:
                                               half * 512 + 512])
                            ablk.append((ab, zw))
                    # yT[d, qblk] = sum_kc v_kc.T @ attT_kc   (N=512)
                    py = ps_z.tile([128, 512], F32, tag="z")
                    for kc in range(nkc):
                        ab, zw = ablk[kc]
                        nc.tensor.matmul(
                            py[:, zw:],
                            vt[kc][:, j * 128:(j + 1) * 128],
                            ab[:, zw:],
                            start=(kc == 0), stop=(kc == nkc - 1),
                            skip_group_check=True)
                    # copy to ytr with fused stats accumulation (ACT)
                    nc.scalar.activation(
                        ytr[:, qb * 512:(qb + 1) * 512], py[:], AF.Copy,
                        accum_out=s1p[:, qb:qb + 1])
                    ysq = wp.tile([128, 512], BF16, tag="ysq", bufs=1)
                    nc.scalar.activation(
                        ysq[:], py[:], AF.Square,
                        accum_out=s2p[:, qb:qb + 1])

                if stage == 2:
                    continue
                if stage == 3:
                    if j == 0:
                        dbg_t = wp.tile([128, T], F32, tag="dbg_t")
                        nc.vector.tensor_copy(dbg_t[:], ytr[:])
                        nc.sync.dma_start(dbg_d, dbg_t[:])
                    continue
                # ---- GroupNorm stats ----
                if stage == 41:
                    if j == 0:
                        dbg_t = wp.tile([128, T], F32, tag="dbg_t")
                        nc.vector.tensor_copy(dbg_t[:], ytr[:])
                        nc.sync.dma_start(dbg_d, dbg_t[:])
                    continue
                s12 = wp.tile([128, 2], F32, tag="s12")
                nc.vector.tensor_reduce(s12[:, 0:1], s1p[:, 0:NT // 4],
                                        axis=mybir.AxisListType.X, op=ALU.add)
                nc.vector.tensor_reduce(s12[:, 1:2], s2p[:, 0:NT // 4],
                                        axis=mybir.AxisListType.X, op=ALU.add)
                if stage == 42:
                    if j == 0:
                        dbg_t = wp.tile([128, T], F32, tag="dbg_t")
                        nc.vector.tensor_copy(dbg_t[:, 0:2], s12[:])
                        nc.sync.dma_start(dbg_d[:, 0:2], dbg_t[:, 0:2])
                    continue
                pg = ps_z.tile([128, 2], F32, tag="z")
                nc.tensor.matmul(pg[:], gg_t[:], s12[:], start=True, stop=True)
                if stage == 43:
                    if j == 0:
                        dbg_t = wp.tile([128, T], F32, tag="dbg_t")
                        nc.vector.tensor_copy(dbg_t[:, 0:2], pg[:])
                        nc.sync.dma_start(dbg_d[:, 0:2], dbg_t[:, 0:2])
                    continue
                # mneg = -mean; nvar = mean^2 - E[y^2] = -var
                mneg = wp.tile([128, 1], F32, tag="mneg")
                nc.scalar.mul(mneg[:], pg[:, 0:1], -1.0 / (T * 4))
                msq = wp.tile([128, 1], F32, tag="msq")
                nc.scalar.mul(msq[:], pg[:, 1:2], 1.0 / (T * 4))
                nvar = wp.tile([128, 1], F32, tag="nvar")
                nc.vector.scalar_tensor_tensor(
                    nvar[:], mneg[:], mneg[:, 0:1], msq[:],
                    op0=ALU.mult, op1=ALU.subtract)
                vpe = wp.tile([128, 1], F32, tag="vpe")
                nc.vector.tensor_scalar(vpe[:], nvar[:], -1.0, EPS,
                                        op0=ALU.mult, op1=ALU.add)  # var+eps
                lnv = wp.tile([128, 1], F32, tag="lnv")
                nc.scalar.activation(lnv[:], vpe[:], AF.Ln)
                rstd = wp.tile([128, 1], F32, tag="rstd")
                nc.scalar.activation(rstd[:], lnv[:], AF.Exp,
                                     scale=-0.5)  # rsqrt(var+eps)
                aff_a = wp.tile([128, 1], F32, tag="aff_a")
                nc.vector.tensor_tensor(aff_a[:], rstd[:], gw2_t[:], ALU.mult)
                aff_b = wp.tile([128, 1], F32, tag="aff_b")
                nc.vector.scalar_tensor_tensor(
                    aff_b[:], mneg[:], aff_a[:, 0:1], gb2_t[:],
                    op0=ALU.mult, op1=ALU.add)  # gb2 - mean*aff_a
                nc.scalar.activation(yt[j][:], ytr[:], AF.Identity,
                                     scale=aff_a[:], bias=aff_b[:])
                if stage == 4 and j == 0:
                    dbg_t = wp.tile([128, T], F32, tag="dbg_t")
                    nc.vector.tensor_copy(dbg_t[:], yt[0][:])
                    nc.sync.dma_start(dbg_d, dbg_t[:])

            # ================= output projection =================
            for ocb in range(8 if stage == 5 else 0):
                for tb in range(T // 512):
                    par = (ocb * (T // 512) + tb) % 2
                    if par == 0:
                        po_ = ps_z.tile([128, 512], F32, tag="z")
                    else:
                        po_ = ps_s.tile([128, 1024], F32, tag="s")
                    for j in range(NH):
                        nc.tensor.matmul(
                            po_[:, 0:512],
                            wct[j][:, ocb * 128:(ocb + 1) * 128],
                            yt[j][:, tb * 512:(tb + 1) * 512],
                            start=(j == 0), stop=(j == NH - 1))
                    ob = wp.tile([128, 512], F32, tag="ob")
                    if par == 0:
                        nc.vector.tensor_copy(ob[:], po_[:, 0:512])
                    else:
                        nc.scalar.copy(ob[:], po_[:, 0:512])
                    nc.sync.dma_start(
                        outT_d[ocb * 128:(ocb + 1) * 128,
                               tb * 512:(tb + 1) * 512], ob[:])
            wp_cm.__exit__(None, None, None)

    nc.compile()
    return nc


def _prep_inputs(inputs):
    bf = ml_dtypes.bfloat16
    x = np.asarray(inputs["x"], np.float32)
    Wq1 = np.asarray(inputs["Wq1"], np.float32)
    Wq2 = np.asarray(inputs["Wq2"], np.float32)
    Wk1 = np.asarray(inputs["Wk1"], np.float32)
    Wk2 = np.asarray(inputs["Wk2"], np.float32)
    Wv = np.asarray(inputs["Wv"], np.float32)
    Wc = np.asarray(inputs["Wc"], np.float32)
    gn_w = np.asarray(inputs["gn_w"], np.float32)
    gn_b = np.asarray(inputs["gn_b"], np.float32)
    gamma = np.asarray(inputs["gamma"], np.float32)

    def sig(v):
        return 1.0 / (1.0 + np.exp(-v))

    lam = (sig(np.asarray(inputs["lq1"], np.float32).reshape(H)
               * np.asarray(inputs["lk1"], np.float32).reshape(H))
           - sig(np.asarray(inputs["lq2"], np.float32).reshape(H)
                 * np.asarray(inputs["lk2"], np.float32).reshape(H))
           + LAMBDA_INIT)

    mask = np.where(np.arange(128)[None, :] <= np.arange(128)[:, None],
                    0.0, NEG).astype(np.float32)
    ident = np.eye(128, dtype=np.float32).astype(bf)
    gg = (np.arange(128)[:, None] // 4 == np.arange(128)[None, :] // 4
          ).astype(np.float32)
    c1 = 1.0 - LAMBDA_INIT
    gw2 = (gn_w * gamma * c1).astype(np.float32).reshape(128, 1)
    gb2 = (gn_b * gamma * c1).astype(np.float32).reshape(128, 1)

    xTb = [np.ascontiguousarray(x[b].T).astype(bf) for b in range(B)]
    in_maps = []
    for core in range(N_CORES):
        b, hg = divmod(core, N_CORES // B)
        qs = hg * NH * HS          # 256-wide q/k slice
        vs = hg * NH * D           # 512-wide v / y2 slice
        lamn = np.repeat(-lam[hg * NH:(hg + 1) * NH].reshape(1, NH),
                         128, axis=0).astype(np.float32)
        in_maps.append({
            "xT": xTb[b],
            "wq1T": np.ascontiguousarray(Wq1[qs:qs + NH * HS, :].T).astype(bf),
            "wq2T": np.ascontiguousarray(Wq2[qs:qs + NH * HS, :].T).astype(bf),
            "wk1T": np.ascontiguousarray(Wk1[qs:qs + NH * HS, :].T).astype(bf),
            "wk2T": np.ascontiguousarray(Wk2[qs:qs + NH * HS, :].T).astype(bf),
            "wvT": np.ascontiguousarray(Wv[vs:vs + NH * D, :].T).astype(bf),
            "wcT": np.ascontiguousarray(Wc[:, vs:vs + NH * D].T).astype(bf),
            "mask128": mask,
            "ident": ident,
            "gg": gg,
            "gw2": gw2,
            "gb2": gb2,
            "lamn": lamn,
        })
    return in_maps


def kernel(**inputs):
    if "nc" not in _cache:
        _cache["nc"] = _build()
    nc = _cache["nc"]
    in_maps = _prep_inputs(inputs)
    res = bass_utils.run_bass_kernel_spmd(
        nc, in_maps, core_ids=list(range(N_CORES)),
        **_cache.get("run_kwargs", {}))
    _cache["last_result"] = res
    out = np.zeros((B, T, C), np.float32)
    for core in range(N_CORES):
        b = core // (N_CORES // B)
        out[b] += res.results[core]["outT"].T
    return out
